# revision 24
# baseline (speedup 1.0000x reference)
"""Trainium2 Bass kernel for nn_Attention (dense transformer block:
QKV proj + RoPE + causal GQA attention + o_proj), SPMD over 8 NeuronCores.

Sharding: core c -> (batch b = c//4, head-group g = c%4). Each core computes
4 query heads + its kv head for one batch, then the head outputs are
AllGather'd within the 4-core batch group and each core computes a disjoint
512-column slice of the o_proj output.

All matmuls run as float32r (TF32-like, ~4x faster than plain fp32 on the
PE). The attention outputs cross the collective in bf16 (halves CC + DMA
traffic); o_proj runs bf16 x bf16. Weights and trig tables are loaded into
SBUF once, outside the rep loop (resident-weights serving model).
"""

import sys
import time

sys.path.insert(0, "/opt/trn_rl_repo")

import numpy as np
import ml_dtypes

import concourse.bass as bass
import concourse.mybir as mybir
import concourse.tile as tile
from concourse import bacc
from concourse.masks import make_identity

F32 = mybir.dt.float32
F32R = mybir.dt.float32r
BF16 = mybir.dt.bfloat16
F16 = mybir.dt.float16
P = 128
HD = 128            # head dim
NHL = 4             # query heads per core
E = 2048            # hidden
DQ = NHL * HD       # 512, local q-projection width / o-slice width
SCALE = 1.0 / np.sqrt(np.float32(HD))
REPLICA_GROUPS = [[0, 1, 2, 3], [4, 5, 6, 7]]
NO_COLLECTIVE = False  # replace AllGather with a local DMA (timeline-sim only)
AG_HALVES = 2          # AllGathers per head (1, 2, or 4; must divide NQC)
DEN_LAG = True         # lag the denominator matmul like pv (hides exp+mask)


def r32(ap):
    return ap.bitcast(F32R)


def build_program(S=2048, reps=1, n_cores=8):
    """Build the per-core SPMD Bass program. Returns compiled nc."""
    ST = S // P          # 128-row tiles along sequence
    NQC = S // 512       # 512-wide chunks along sequence
    ET = E // P          # 16 tiles along hidden

    nc = bacc.Bacc("TRN2", target_bir_lowering=False, debug=False,
                   num_devices=n_cores)

    x_in = nc.declare_dram_parameter("xT", [E, S], BF16, isOutput=False)
    wqT_in = nc.declare_dram_parameter("wqT", [E, DQ], BF16, isOutput=False)
    wkT_in = nc.declare_dram_parameter("wkT", [E, HD], BF16, isOutput=False)
    wvT_in = nc.declare_dram_parameter("wvT", [E, HD], BF16, isOutput=False)
    woT_in = nc.declare_dram_parameter("woT", [E, DQ], BF16, isOutput=False)
    cosT_in = nc.declare_dram_parameter("cosT", [HD, S], F32, isOutput=False)
    sinT_in = nc.declare_dram_parameter("sinT", [HD, S], F32, isOutput=False)
    out_d = nc.declare_dram_parameter("out", [DQ, S], F32, isOutput=True)

    with tile.TileContext(nc) as tc:
        with nc.allow_low_precision(reason="float32r/bf16 rounding for PE operands"):
            _emit(tc, nc, S, ST, NQC, ET, reps,
                  x_in, wqT_in, wkT_in, wvT_in, woT_in, cosT_in, sinT_in, out_d)

    nc.compile()
    return nc


def _emit(tc, nc, S, ST, NQC, ET, reps,
          x_in, wqT_in, wkT_in, wvT_in, woT_in, cosT_in, sinT_in, out_d):
    from contextlib import ExitStack

    ctx = ExitStack()
    with ctx:
        const = ctx.enter_context(tc.tile_pool(name="const", bufs=1))
        wper = ctx.enter_context(tc.tile_pool(name="wper", bufs=1))
        qkv = ctx.enter_context(tc.tile_pool(name="qkv", bufs=1))
        dram = ctx.enter_context(tc.tile_pool(name="dram", bufs=1, space="DRAM"))
        af_pool = ctx.enter_context(tc.tile_pool(name="af", bufs=3))
        acc_pool = ctx.enter_context(tc.tile_pool(name="acc", bufs=1))
        oo_ps = ctx.enter_context(tc.tile_pool(name="oo_ps", bufs=2, space="PSUM"))

        # ---- constants ----
        ident = const.tile([P, P], F32)
        make_identity(nc, ident[:])
        identr_t = const.tile([P, P], F32R)
        nc.vector.tensor_copy(identr_t[:], ident[:])
        masks = const.tile([P, 4 * 512], F16)
        nc.gpsimd.memset(masks[:], 1.0)
        for t in range(4):
            # valid(k_local, q_local) = (q_local - k_local - 128*t) >= 0
            nc.gpsimd.affine_select(
                out=masks[:, t * 512:(t + 1) * 512],
                in_=masks[:, t * 512:(t + 1) * 512],
                compare_op=mybir.AluOpType.is_ge,
                fill=0.0, base=-P * t, pattern=[[1, 512]],
                channel_multiplier=-1,
            )
        ones_stage = const.tile([P, P], F32)
        nc.gpsimd.memset(ones_stage[:], 1.0)
        ones_red = const.tile([P, 1], F16)
        nc.vector.tensor_copy(ones_red[:], ones_stage[:, 0:1])
        ones_col = const.tile([1, P], F32R)
        nc.vector.tensor_copy(ones_col[:], ones_stage[0:1, :])

        # ---- persistent weights (loaded once, shared by all reps) ----
        wqT_sb = wper.tile([P, ET, DQ], BF16)
        wkT_sb = wper.tile([P, ET, HD], BF16)
        wvT_sb = wper.tile([P, ET, HD], BF16)
        woT_sb = wper.tile([P, ET, DQ], BF16)
        wq_r = wqT_in.rearrange("(et p) d -> p et d", p=P)
        wk_r = wkT_in.rearrange("(et p) d -> p et d", p=P)
        wv_r = wvT_in.rearrange("(et p) d -> p et d", p=P)
        wo_r = woT_in.rearrange("(et p) d -> p et d", p=P)
        for et in range(ET):
            nc.sync.dma_start(wqT_sb[:, et, :], wq_r[:, et, :])
            nc.sync.dma_start(wkT_sb[:, et, :], wk_r[:, et, :])
            nc.sync.dma_start(wvT_sb[:, et, :], wv_r[:, et, :])
        nc.sync.dma_start(woT_sb[:], wo_r)

        # ---- persistent SBUF ----
        QT_sb = qkv.tile([P, NHL, S], BF16)
        KT_sb = qkv.tile([P, S], BF16)
        V_sb = qkv.tile([P, ST, HD], F16)

        # collective bounce buffers (DRAM), one per (head, seq-half), bf16
        NHALF = min(AG_HALVES, NQC)
        SH = S // NHALF
        agin = [[dram.tile([P, SH], BF16, name=f"agin{h}_{hf}")
                 for hf in range(NHALF)] for h in range(NHL)]
        agout = [[dram.tile([4 * P, SH], BF16, name=f"agout{h}_{hf}")
                  for hf in range(NHALF)] for h in range(NHL)]

        outAcc = acc_pool.tile([P, 4, S], F32)

        deferred = None
        for rep in range(reps):
            deferred = _emit_rep(tc, nc, S, ST, NQC, ET, ctx, rep,
                                 x_in, cosT_in, sinT_in,
                                 out_d, identr_t, masks, ones_red, ones_col,
                                 wqT_sb, wkT_sb, wvT_sb, woT_sb,
                                 QT_sb, KT_sb, V_sb, agin, agout,
                                 af_pool, oo_ps, outAcc, deferred)
        if deferred is not None:
            deferred["af"]()
            deferred["mm"]()


def _oproj_groups(nc, h, cph, NHALF, agout, af_pool, oo_ps, woT_sb,
                  outAcc, out_d, hf):
    """One closure per (sch, ot) o_proj matmul group; the ot==0 closure of
    each seq chunk also issues its af DMA. Popped one-per-kt-tile so the
    PE stays dense while the kt loop is Act(exp)-paced."""
    out_r = out_d.rearrange("(ot p) s -> p ot s", p=P)
    ag_r = agout[h][hf].rearrange("(mt p) s -> p mt s", p=P)
    groups = []
    for sch in range(cph):
        state = {}
        for ot in range(4):
            def g(sch=sch, ot=ot, state=state):
                s0 = (hf * cph + sch) * 512
                if ot == 0:
                    af = af_pool.tile([P, 4, 512], BF16, name="af", tag="af")
                    nc.sync.dma_start(
                        af[:], ag_r[:, :, sch * 512:(sch + 1) * 512])
                    state["af"] = af
                af = state["af"]
                po = oo_ps.tile([P, 512], F32, name="po", tag="po")
                for mt in range(4):
                    nc.tensor.matmul(
                        po[:],
                        woT_sb[:, 4 * mt + h, ot * P:(ot + 1) * P],
                        af[:, mt, :],
                        start=(mt == 0), stop=(mt == 3))
                acc = outAcc[:, ot, s0:s0 + 512]
                if h == 0:
                    if ot % 2 == 0:
                        nc.scalar.copy(acc, po[:])
                    else:
                        nc.vector.tensor_copy(acc, po[:])
                else:
                    nc.vector.tensor_add(acc, acc, po[:])
                if h == NHL - 1:
                    nc.sync.dma_start(out_r[:, ot, s0:s0 + 512], acc)
            groups.append(g)
    return groups


def _emit_oproj(nc, h, NQC, cph, NHALF, agout, af_pool, oo_ps, woT_sb,
                outAcc, out_d, only_hf=None):
    for hf in range(NHALF):
        if only_hf is not None and hf != only_hf:
            continue
        for g in _oproj_groups(nc, h, cph, NHALF, agout, af_pool, oo_ps,
                               woT_sb, outAcc, out_d, hf):
            g()


def _emit_rep(tc, nc, S, ST, NQC, ET, ctx, rep,
              x_in, cosT_in, sinT_in,
              out_d, ident, masks, ones_red, ones_col,
              wqT_sb, wkT_sb, wvT_sb, woT_sb,
              QT_sb, KT_sb, V_sb, agin, agout,
              af_pool, oo_ps, outAcc, deferred):
    from contextlib import ExitStack

    identr = ident[:]  # pre-rounded F32R identity

    # ================= projection phase =================
    with ExitStack() as pctx:
        trig_pool = pctx.enter_context(tc.tile_pool(name="trig", bufs=1))
        xt_pool = pctx.enter_context(tc.tile_pool(name="xt", bufs=18))
        rope_pool = pctx.enter_context(tc.tile_pool(name="rope", bufs=2))
        vt_pool = pctx.enter_context(tc.tile_pool(name="vt", bufs=2))
        pt_ps = pctx.enter_context(tc.tile_pool(name="pt_ps", bufs=2, space="PSUM"))
        pj_ps = pctx.enter_context(tc.tile_pool(name="pj_ps", bufs=4, space="PSUM"))

        cosT_sb = trig_pool.tile([P, S], F32)
        sinT_sb = trig_pool.tile([P, S], F32)

        x_r = x_in.rearrange("(et p) s -> p et s", p=P)

        for sc in range(NQC):
            s0 = sc * 512
            # x^T tiles of this s-chunk arrive pre-transposed from the host
            xts = []
            for et in range(ET):
                xt_t = xt_pool.tile([P, 512], BF16, name="xts", tag="xts")
                nc.sync.dma_start(xt_t[:], x_r[:, et, s0:s0 + 512])
                xts.append(xt_t)
            if sc == 0:
                nc.sync.dma_start(cosT_sb[:], cosT_in[:])
                nc.sync.dma_start(sinT_sb[:], sinT_in[:])
                if deferred is not None:
                    deferred["af"]()
            if sc == 2 and deferred is not None:
                deferred["mm"]()
                deferred = None

            # d6-outer matmul loop over resident xts tiles
            cos_c = cosT_sb[:, s0:s0 + 512]
            sin_c = sinT_sb[:, s0:s0 + 512]
            for d6 in range(6):
                pp = pj_ps.tile([P, 512], F32, name="pp", tag="pp")
                for et in range(ET):
                    if d6 < 4:
                        lhsT = wqT_sb[:, et, d6 * HD:(d6 + 1) * HD]
                    elif d6 == 4:
                        lhsT = wkT_sb[:, et, :]
                    else:
                        lhsT = wvT_sb[:, et, :]
                    nc.tensor.matmul(pp[:], lhsT, xts[et][:],
                                     start=(et == 0), stop=(et == ET - 1))
                if d6 < 5:
                    dst = (QT_sb[:, d6, s0:s0 + 512] if d6 < 4
                           else KT_sb[:, s0:s0 + 512])
                    t1 = rope_pool.tile([P, 512], F32, name="t1", tag="t1")
                    t2 = rope_pool.tile([P, 512], F32, name="t2", tag="t2")
                    nc.vector.tensor_tensor(t1[:], pp[:], cos_c,
                                            mybir.AluOpType.mult)
                    # sinT arrives with rows 0:64 pre-negated (host side)
                    nc.vector.tensor_tensor(t2[0:64, :], pp[64:128, :],
                                            sin_c[0:64, :],
                                            mybir.AluOpType.mult)
                    nc.vector.tensor_tensor(t2[64:128, :], pp[0:64, :],
                                            sin_c[64:128, :],
                                            mybir.AluOpType.mult)
                    nc.vector.tensor_tensor(dst[:], t1[:], t2[:],
                                            mybir.AluOpType.add)
                else:
                    vts = vt_pool.tile([P, 512], F32R, name="vts", tag="vts")
                    nc.scalar.copy(vts[:], pp[:])
                    for st4 in range(4):
                        pv_t = pt_ps.tile([P, 512], F32, name="pvt",
                                          tag="ptile")[:, 0:P]
                        nc.tensor.transpose(r32(pv_t[:]),
                                            vts[:, st4 * P:(st4 + 1) * P],
                                            identr)
                        nc.scalar.copy(V_sb[:, sc * 4 + st4, :], pv_t[:])

    # ================= attention + o_proj phase =================
    with ExitStack() as actx:
        ex_pool = actx.enter_context(tc.tile_pool(name="ex", bufs=8))
        dn_pool = actx.enter_context(tc.tile_pool(name="dn", bufs=2))
        sm_pool = actx.enter_context(tc.tile_pool(name="sm", bufs=2))
        bc_pool = actx.enter_context(tc.tile_pool(name="bc", bufs=2))
        oh_pool = actx.enter_context(tc.tile_pool(name="oh", bufs=3))
        sc_ps = actx.enter_context(tc.tile_pool(name="sc_ps", bufs=3, space="PSUM"))
        pv_ps = actx.enter_context(tc.tile_pool(name="pv_ps", bufs=2, space="PSUM"))
        dn_ps = actx.enter_context(tc.tile_pool(name="dn_ps", bufs=1, space="PSUM"))

        NHALF = min(AG_HALVES, NQC)
        cph = NQC // NHALF

        def emit_ag(h, hf):
            if NO_COLLECTIVE:
                for mt in range(4):
                    nc.sync.dma_start(
                        agout[h][hf][mt * P:(mt + 1) * P, :], agin[h][hf][:])
            else:
                nc.gpsimd.collective_compute(
                    "AllGather", mybir.AluOpType.bypass,
                    replica_groups=REPLICA_GROUPS,
                    ins=[agin[h][hf].opt()],
                    outs=[agout[h][hf].opt()])

        bg = []  # background o_proj groups interleaved into kt slots
        for h in range(NHL):
            if h > 0:
                bg += _oproj_groups(nc, h - 1, cph, NHALF, agout, af_pool,
                                    oo_ps, woT_sb, outAcc, out_d, hf=0)
            for qc in range(NQC):
                q0 = qc * 512
                nkt = 4 * qc + 4
                qT = QT_sb[:, h, q0:q0 + 512]
                pv = pv_ps.tile([P, 512], F32, name="pv", tag="pv")
                denacc = dn_pool.tile([P, 512], F16, name="denacc",
                                      tag="denacc")
                exs = [None] * nkt
                LAG = 4  # pv(kt-LAG) emitted after scores(kt): hides exp+mask
                def emit_pv(j, last):
                    nc.tensor.matmul(pv[:], V_sb[:, j, :], exs[j][:],
                                     start=(j == 0), stop=last)
                for kt in range(nkt):
                    ps = sc_ps.tile([P, 512], F32, name="ps", tag="ps")
                    nc.tensor.matmul(
                        ps[:], KT_sb[:, kt * P:(kt + 1) * P], qT,
                        start=True, stop=True)
                    ex = ex_pool.tile([P, 512], F16, name="ex", tag="ex")
                    exs[kt] = ex
                    nc.scalar.activation(ex[:], ps[:],
                                         mybir.ActivationFunctionType.Exp,
                                         scale=float(SCALE))
                    t = kt - 4 * qc
                    if t >= 0:
                        nc.vector.tensor_tensor(
                            ex[:], ex[:],
                            masks[:, t * 512:(t + 1) * 512],
                            mybir.AluOpType.mult)
                    # denominator: fp16 adds on DVE (frees the PE)
                    if kt == 0:
                        nc.vector.tensor_copy(denacc[:], ex[:])
                    else:
                        nc.vector.tensor_add(denacc[:], denacc[:], ex[:])
                    if kt >= LAG:
                        emit_pv(kt - LAG, last=False)
                    if h > 0 and qc == NQC - 1 and kt == 4:
                        # previous head's second-half AllGather has now had
                        # ~3 q-chunks of attention to land
                        bg += _oproj_groups(nc, h - 1, cph, NHALF, agout,
                                            af_pool, oo_ps, woT_sb, outAcc,
                                            out_d, hf=1)
                    if bg:
                        bg.pop(0)()
                for j in range(max(0, nkt - LAG), nkt):
                    emit_pv(j, last=(j == nkt - 1))
                # partition-reduce the fp16 denominator on the PE (1 matmul)
                pden = dn_ps.tile([P, 512], F32, name="pden", tag="pden")
                nc.tensor.matmul(pden[0:1, :], ones_red[:], denacc[:],
                                 start=True, stop=True)
                rec = sm_pool.tile([1, 512], F32, name="rec", tag="rec")
                nc.vector.reciprocal_approx_fast(out=rec[:], in_=pden[0:1, :])
                recr = sm_pool.tile([1, 512], F32R, name="recr", tag="recr")
                nc.vector.tensor_copy(recr[:], rec[:])
                pbc = sc_ps.tile([P, 512], F32, name="pbc", tag="ps")
                nc.tensor.matmul(pbc[:], ones_col[:], recr[:],
                                 start=True, stop=True)
                bcr = bc_pool.tile([P, 512], F32, name="bcr", tag="bcr")
                nc.vector.tensor_copy(bcr[:], pbc[:])
                outH = oh_pool.tile([P, 512], BF16, name="outH", tag="outH")
                nc.vector.tensor_tensor(outH[:], pv[:], bcr[:],
                                        mybir.AluOpType.mult)
                hf = qc // cph
                qh0 = (qc - hf * cph) * 512
                nc.sync.dma_start(agin[h][hf][:, qh0:qh0 + 512], outH[:])
                # ship this half as soon as its last q-chunk is done;
                # then emit the o_proj half of the previous head whose
                # AllGather has had a full half-head of attention to land
                if qc % cph == cph - 1:
                    emit_ag(h, hf)
        while bg:
            bg.pop(0)()
        _emit_oproj(nc, NHL - 1, NQC, cph, NHALF, agout, af_pool,
                    oo_ps, woT_sb, outAcc, out_d, only_hf=0)

    # deferred tail: af DMAs issued early (gpsimd queue, idle during proj)
    # so the read-back never contends with the next rep's x loads; matmuls
    # pop two proj chunks later when the data has surely landed.
    hf_last = NHALF - 1
    ag_r_last = agout[NHL - 1][hf_last].rearrange("(mt p) s -> p mt s", p=P)
    out_r_last = out_d.rearrange("(ot p) s -> p ot s", p=P)
    state = {}

    def deferred_af():
        for sch in range(cph):
            af = af_pool.tile([P, 4, 512], BF16, name="af", tag="af")
            nc.gpsimd.dma_start(
                af[:], ag_r_last[:, :, sch * 512:(sch + 1) * 512])
            state[sch] = af

    def deferred_mm():
        for sch in range(cph):
            s0 = (hf_last * cph + sch) * 512
            af = state[sch]
            for ot in range(4):
                po = oo_ps.tile([P, 512], F32, name="po", tag="po")
                for mt in range(4):
                    nc.tensor.matmul(
                        po[:],
                        woT_sb[:, 4 * mt + (NHL - 1), ot * P:(ot + 1) * P],
                        af[:, mt, :],
                        start=(mt == 0), stop=(mt == 3))
                acc = outAcc[:, ot, s0:s0 + 512]
                nc.vector.tensor_add(acc, acc, po[:])
                nc.sync.dma_start(out_r_last[:, ot, s0:s0 + 512], acc)

    return {"af": deferred_af, "mm": deferred_mm}


# ======================= host side =======================

_CACHE = {}


def _get_program(S=2048, reps=1):
    key = (S, reps, AG_HALVES, NO_COLLECTIVE, DEN_LAG)
    if key not in _CACHE:
        _CACHE[key] = build_program(S=S, reps=reps)
    return _CACHE[key]


def make_in_maps(x, cos, sin, wq, wk, wv, wo):
    in_maps = []
    cosT = np.ascontiguousarray(cos.T.astype(np.float32))
    sinT = sin.T.astype(np.float32).copy()
    sinT[:HD // 2, :] *= -1.0   # fold rotate_half sign into the table
    sinT = np.ascontiguousarray(sinT)
    for c in range(8):
        b, g = c // 4, c % 4
        in_maps.append({
            "xT": np.ascontiguousarray(x[b].T.astype(ml_dtypes.bfloat16)),
            "wqT": np.ascontiguousarray(wq[g * DQ:(g + 1) * DQ, :].T.astype(ml_dtypes.bfloat16)),
            "wkT": np.ascontiguousarray(wk[g * HD:(g + 1) * HD, :].T.astype(ml_dtypes.bfloat16)),
            "wvT": np.ascontiguousarray(wv[g * HD:(g + 1) * HD, :].T.astype(ml_dtypes.bfloat16)),
            "woT": np.ascontiguousarray(
                wo[g * DQ:(g + 1) * DQ, :].T.astype(ml_dtypes.bfloat16)),
            "cosT": cosT,
            "sinT": sinT,
        })
    return in_maps


def assemble_output(results, B, S):
    out = np.empty((B, S, E), np.float32)
    for c in range(8):
        b, g = c // 4, c % 4
        out[b][:, g * DQ:(g + 1) * DQ] = results[c]["out"].T
    return out


# ---- inline SPMD runner (PJRT/axon), device-resident inputs ----

class SpmdRunner:
    def __init__(self, nc, n_cores):
        import jax
        from jax.sharding import Mesh, PartitionSpec
        from jax.experimental.shard_map import shard_map
        from concourse import bass2jax
        from concourse.bass2jax import _bass_exec_p, install_neuronx_cc_hook

        install_neuronx_cc_hook()
        self.jax = jax
        self.nc = nc
        self.n_cores = n_cores
        partition_name = (nc.partition_id_tensor.name
                          if nc.partition_id_tensor else None)
        in_names, out_names, out_avals = [], [], []
        zero_outs = []
        for alloc in nc.m.functions[0].allocations:
            if not isinstance(alloc, mybir.MemoryLocationSet):
                continue
            name = alloc.memorylocations[0].name
            if alloc.kind == "ExternalInput":
                if name != partition_name:
                    in_names.append(name)
            elif alloc.kind == "ExternalOutput":
                out_names.append(name)
                shape = tuple(alloc.tensor_shape)
                dtype = mybir.dt.np(alloc.dtype)
                out_avals.append(jax.core.ShapedArray(shape, dtype))
                zero_outs.append(np.zeros(shape, dtype))
        self.in_names, self.out_names = in_names, out_names
        self.out_avals, self.zero_outs = out_avals, zero_outs
        self.n_params = len(in_names)

        all_in = list(in_names) + list(out_names)
        if partition_name is not None:
            all_in.append(partition_name)

        def _body(*args):
            operands = list(args)
            if partition_name is not None:
                operands.append(bass2jax.partition_id_tensor())
            outs = _bass_exec_p.bind(
                *operands, out_avals=tuple(out_avals),
                in_names=tuple(all_in), out_names=tuple(out_names),
                lowering_input_output_aliases=(),
                sim_require_finite=True, sim_require_nnan=True, nc=nc)
            return tuple(outs)

        devices = jax.devices()[:n_cores]
        self.mesh = Mesh(np.asarray(devices), ("core",))
        n_outs = len(out_names)
        in_specs = (PartitionSpec("core"),) * (self.n_params + n_outs)
        out_specs = (PartitionSpec("core"),) * n_outs
        self.fn = jax.jit(
            shard_map(_body, mesh=self.mesh, in_specs=in_specs,
                      out_specs=out_specs, check_rep=False),
            keep_unused=True)
        self.dev_args = None

    def stage_inputs(self, in_maps):
        import jax
        from jax.sharding import PartitionSpec
        per_core = [[np.asarray(m[n]) for n in self.in_names] for m in in_maps]
        concat_in = [
            np.concatenate([per_core[c][i] for c in range(self.n_cores)], axis=0)
            for i in range(self.n_params)]
        concat_zeros = [
            np.zeros((self.n_cores * z.shape[0], *z.shape[1:]), z.dtype)
            for z in self.zero_outs]
        sharding = jax.sharding.NamedSharding(self.mesh, PartitionSpec("core"))
        self.dev_args = [jax.device_put(a, sharding)
                         for a in (*concat_in, *concat_zeros)]
        for a in self.dev_args:
            a.block_until_ready()

    def run(self):
        out_arrs = [np.asarray(o) for o in self.fn(*self.dev_args)]
        return [
            {n: out_arrs[i].reshape(self.n_cores, *self.out_avals[i].shape)[c]
             for i, n in enumerate(self.out_names)}
            for c in range(self.n_cores)]

    def time_exec(self, iters=30, warmup=3):
        import jax
        for _ in range(warmup):
            res = self.fn(*self.dev_args)
        jax.block_until_ready(res)
        t0 = time.perf_counter()
        for _ in range(iters):
            res = self.fn(*self.dev_args)
        jax.block_until_ready(res)
        t1 = time.perf_counter()
        return (t1 - t0) / iters * 1e9


_RUNNER_CACHE = {}


def get_runner(S=2048, reps=1):
    key = (S, reps, AG_HALVES, NO_COLLECTIVE, DEN_LAG)
    if key not in _RUNNER_CACHE:
        nc = _get_program(S=S, reps=reps)
        _RUNNER_CACHE[key] = SpmdRunner(nc, 8)
    return _RUNNER_CACHE[key]


def kernel(x, cos, sin, wq, wk, wv, wo):
    B, S, _ = x.shape
    runner = get_runner(S=S, reps=1)
    runner.stage_inputs(make_in_maps(x, cos, sin, wq, wk, wv, wo))
    results = runner.run()
    return assemble_output(results, B, S)


if __name__ == "__main__":
    # tiny self-test against a local numpy reference
    S = int(sys.argv[1]) if len(sys.argv) > 1 else 512
    rng = np.random.default_rng(0)
    B, H, HKV = 2, 16, 4
    x = rng.standard_normal((B, S, E), dtype=np.float32)
    cos = rng.random((S, HD), dtype=np.float32)
    sin = rng.random((S, HD), dtype=np.float32)
    sc = 0.02
    wq = (rng.standard_normal((H * HD, E), dtype=np.float32) * sc)
    wk = (rng.standard_normal((HKV * HD, E), dtype=np.float32) * sc)
    wv = (rng.standard_normal((HKV * HD, E), dtype=np.float32) * sc)
    wo = (rng.standard_normal((E, H * HD), dtype=np.float32) * sc)

    def ref(x, cos, sin, wq, wk, wv, wo):
        x64 = x.astype(np.float64)
        q = (x64 @ wq.T.astype(np.float64)).reshape(B, S, H, HD)
        k = (x64 @ wk.T.astype(np.float64)).reshape(B, S, HKV, HD)
        v = (x64 @ wv.T.astype(np.float64)).reshape(B, S, HKV, HD)

        def rot(t):
            return np.concatenate([-t[..., HD // 2:], t[..., :HD // 2]], -1)

        c = cos[:, None, :].astype(np.float64)
        s = sin[:, None, :].astype(np.float64)
        q = q * c + rot(q) * s
        k = k * c + rot(k) * s
        k = np.repeat(k, H // HKV, axis=2).transpose(0, 2, 1, 3)
        v = np.repeat(v, H // HKV, axis=2).transpose(0, 2, 1, 3)
        q = q.transpose(0, 2, 1, 3)
        scores = np.einsum("bhqd,bhkd->bhqk", q, k) / np.sqrt(HD)
        mask = np.tril(np.ones((S, S), bool))
        scores = np.where(mask, scores, -np.inf)
        scores -= scores.max(-1, keepdims=True)
        p = np.exp(scores)
        p /= p.sum(-1, keepdims=True)
        o = np.einsum("bhqk,bhkd->bhqd", p, v)
        o = o.transpose(0, 2, 1, 3).reshape(B, S, H * HD)
        return o @ wo.T.astype(np.float64)

    want = ref(x, cos, sin, wq, wk, wv, wo)
    got = kernel(x, cos, sin, wq, wk, wv, wo)
    err = np.abs(got - want).max() / np.abs(want).max()
    print(f"S={S}: rel err (absmax-relative) = {err:.3e}")


# revision 26
# speedup vs baseline: 1.2542x; 1.2542x over previous
"""Trainium2 Bass kernel for nn_Attention (dense transformer block:
QKV proj + RoPE + causal GQA attention + o_proj), SPMD over 8 NeuronCores.

Sharding: core c -> (batch b = c//4, head-group g = c%4). Each core computes
4 query heads + its kv head for one batch, then the head outputs are
AllGather'd within the 4-core batch group and each core computes a disjoint
512-column slice of the o_proj output.

All matmuls run as float32r (TF32-like, ~4x faster than plain fp32 on the
PE). The attention outputs cross the collective in bf16 (halves CC + DMA
traffic); o_proj runs bf16 x bf16. Weights and trig tables are loaded into
SBUF once, outside the rep loop (resident-weights serving model).
"""

import sys
import time

sys.path.insert(0, "/opt/trn_rl_repo")

import numpy as np
import ml_dtypes

import concourse.bass as bass
import concourse.mybir as mybir
import concourse.tile as tile
from concourse import bacc
from concourse.masks import make_identity

F32 = mybir.dt.float32
F32R = mybir.dt.float32r
BF16 = mybir.dt.bfloat16
F16 = mybir.dt.float16
P = 128
HD = 128            # head dim
NHL = 4             # query heads per core
E = 2048            # hidden
DQ = NHL * HD       # 512, local q-projection width / o-slice width
SCALE = 1.0 / np.sqrt(np.float32(HD))
REPLICA_GROUPS = [[0, 1, 2, 3], [4, 5, 6, 7]]
NO_COLLECTIVE = False  # replace AllGather with a local DMA (timeline-sim only)
AG_HALVES = 2          # AllGathers per head (1, 2, or 4; must divide NQC)
DEN_LAG = True         # lag the denominator matmul like pv (hides exp+mask)


def r32(ap):
    return ap.bitcast(F32R)


def build_program(S=2048, reps=1, n_cores=8):
    """Build the per-core SPMD Bass program. Returns compiled nc."""
    ST = S // P          # 128-row tiles along sequence
    NQC = S // 512       # 512-wide chunks along sequence
    ET = E // P          # 16 tiles along hidden

    nc = bacc.Bacc("TRN2", target_bir_lowering=False, debug=False,
                   num_devices=n_cores)

    x_in = nc.declare_dram_parameter("xT", [E, S], BF16, isOutput=False)
    wqT_in = nc.declare_dram_parameter("wqT", [E, DQ], BF16, isOutput=False)
    wkT_in = nc.declare_dram_parameter("wkT", [E, HD], BF16, isOutput=False)
    wvT_in = nc.declare_dram_parameter("wvT", [E, HD], BF16, isOutput=False)
    woT_in = nc.declare_dram_parameter("woT", [E, DQ], BF16, isOutput=False)
    cosT_in = nc.declare_dram_parameter("cosT", [HD, S], F32, isOutput=False)
    sinT_in = nc.declare_dram_parameter("sinT", [HD, S], F32, isOutput=False)
    out_d = nc.declare_dram_parameter("out", [DQ, S], F32, isOutput=True)

    with tile.TileContext(nc) as tc:
        with nc.allow_low_precision(reason="float32r/bf16 rounding for PE operands"):
            _emit(tc, nc, S, ST, NQC, ET, reps,
                  x_in, wqT_in, wkT_in, wvT_in, woT_in, cosT_in, sinT_in, out_d)

    nc.compile()
    return nc


def _emit(tc, nc, S, ST, NQC, ET, reps,
          x_in, wqT_in, wkT_in, wvT_in, woT_in, cosT_in, sinT_in, out_d):
    from contextlib import ExitStack

    ctx = ExitStack()
    with ctx:
        const = ctx.enter_context(tc.tile_pool(name="const", bufs=1))
        wper = ctx.enter_context(tc.tile_pool(name="wper", bufs=1))
        qkv = ctx.enter_context(tc.tile_pool(name="qkv", bufs=1))
        dram = ctx.enter_context(tc.tile_pool(name="dram", bufs=1, space="DRAM"))
        af_pool = ctx.enter_context(tc.tile_pool(name="af", bufs=3))
        acc_pool = ctx.enter_context(tc.tile_pool(name="acc", bufs=1))
        oo_ps = ctx.enter_context(tc.tile_pool(name="oo_ps", bufs=2, space="PSUM"))

        # ---- constants ----
        ident = const.tile([P, P], F32)
        make_identity(nc, ident[:])
        identr_t = const.tile([P, P], F32R)
        nc.vector.tensor_copy(identr_t[:], ident[:])
        masks = const.tile([P, 4 * 512], F16)
        nc.gpsimd.memset(masks[:], 1.0)
        for t in range(4):
            # valid(k_local, q_local) = (q_local - k_local - 128*t) >= 0
            nc.gpsimd.affine_select(
                out=masks[:, t * 512:(t + 1) * 512],
                in_=masks[:, t * 512:(t + 1) * 512],
                compare_op=mybir.AluOpType.is_ge,
                fill=0.0, base=-P * t, pattern=[[1, 512]],
                channel_multiplier=-1,
            )
        ones_stage = const.tile([P, P], F32)
        nc.gpsimd.memset(ones_stage[:], 1.0)
        ones_red = const.tile([P, 1], F16)
        nc.vector.tensor_copy(ones_red[:], ones_stage[:, 0:1])
        ones_col = const.tile([1, P], F32R)
        nc.vector.tensor_copy(ones_col[:], ones_stage[0:1, :])

        # ---- persistent weights (loaded once, shared by all reps) ----
        wqT_sb = wper.tile([P, ET, DQ], BF16)
        wkT_sb = wper.tile([P, ET, HD], BF16)
        wvT_sb = wper.tile([P, ET, HD], BF16)
        woT_sb = wper.tile([P, ET, DQ], BF16)
        wq_r = wqT_in.rearrange("(et p) d -> p et d", p=P)
        wk_r = wkT_in.rearrange("(et p) d -> p et d", p=P)
        wv_r = wvT_in.rearrange("(et p) d -> p et d", p=P)
        wo_r = woT_in.rearrange("(et p) d -> p et d", p=P)
        for et in range(ET):
            nc.sync.dma_start(wqT_sb[:, et, :], wq_r[:, et, :])
            nc.sync.dma_start(wkT_sb[:, et, :], wk_r[:, et, :])
            nc.sync.dma_start(wvT_sb[:, et, :], wv_r[:, et, :])
        nc.sync.dma_start(woT_sb[:], wo_r)

        # ---- persistent SBUF ----
        QT_sb = qkv.tile([P, NHL, S], BF16)
        KT_sb = qkv.tile([P, S], BF16)
        V_sb = qkv.tile([P, ST, HD], F16)

        # collective bounce buffers (DRAM), one per (head, seq-half), bf16
        NHALF = min(AG_HALVES, NQC)
        SH = S // NHALF
        agin = [[dram.tile([P, SH], BF16, name=f"agin{h}_{hf}")
                 for hf in range(NHALF)] for h in range(NHL)]
        agout = [[dram.tile([4 * P, SH], BF16, name=f"agout{h}_{hf}")
                  for hf in range(NHALF)] for h in range(NHL)]

        outAcc = acc_pool.tile([P, 4, S], F32)
        xpre = acc_pool.tile([P, ET, 512], BF16)

        deferred = None
        for rep in range(reps):
            deferred = _emit_rep(tc, nc, S, ST, NQC, ET, ctx, rep,
                                 x_in, cosT_in, sinT_in,
                                 out_d, identr_t, masks, ones_red, ones_col,
                                 wqT_sb, wkT_sb, wvT_sb, woT_sb,
                                 QT_sb, KT_sb, V_sb, agin, agout,
                                 af_pool, oo_ps, outAcc, deferred,
                                 xpre, rep > 0, rep < reps - 1)
        if deferred is not None:
            deferred()


def _oproj_groups(nc, h, cph, NHALF, agout, af_pool, oo_ps, woT_sb,
                  outAcc, out_d, hf):
    """One closure per (sch, ot) o_proj matmul group; the ot==0 closure of
    each seq chunk also issues its af DMA. Popped one-per-kt-tile so the
    PE stays dense while the kt loop is Act(exp)-paced."""
    out_r = out_d.rearrange("(ot p) s -> p ot s", p=P)
    ag_r = agout[h][hf].rearrange("(mt p) s -> p mt s", p=P)
    groups = []
    for sch in range(cph):
        state = {}
        for ot in range(4):
            def g(sch=sch, ot=ot, state=state):
                s0 = (hf * cph + sch) * 512
                if ot == 0:
                    af = af_pool.tile([P, 4, 512], BF16, name="af", tag="af")
                    nc.sync.dma_start(
                        af[:], ag_r[:, :, sch * 512:(sch + 1) * 512])
                    state["af"] = af
                af = state["af"]
                po = oo_ps.tile([P, 512], F32, name="po", tag="po")
                for mt in range(4):
                    nc.tensor.matmul(
                        po[:],
                        woT_sb[:, 4 * mt + h, ot * P:(ot + 1) * P],
                        af[:, mt, :],
                        start=(mt == 0), stop=(mt == 3))
                acc = outAcc[:, ot, s0:s0 + 512]
                if h == 0:
                    if ot % 2 == 0:
                        nc.scalar.copy(acc, po[:])
                    else:
                        nc.vector.tensor_copy(acc, po[:])
                else:
                    nc.vector.tensor_add(acc, acc, po[:])
                if h == NHL - 1:
                    nc.sync.dma_start(out_r[:, ot, s0:s0 + 512], acc)
            groups.append(g)
    return groups


def _emit_oproj(nc, h, NQC, cph, NHALF, agout, af_pool, oo_ps, woT_sb,
                outAcc, out_d, only_hf=None):
    for hf in range(NHALF):
        if only_hf is not None and hf != only_hf:
            continue
        for g in _oproj_groups(nc, h, cph, NHALF, agout, af_pool, oo_ps,
                               woT_sb, outAcc, out_d, hf):
            g()


def _emit_rep(tc, nc, S, ST, NQC, ET, ctx, rep,
              x_in, cosT_in, sinT_in,
              out_d, ident, masks, ones_red, ones_col,
              wqT_sb, wkT_sb, wvT_sb, woT_sb,
              QT_sb, KT_sb, V_sb, agin, agout,
              af_pool, oo_ps, outAcc, deferred,
              xpre, use_prefetch, do_prefetch):
    from contextlib import ExitStack

    identr = ident[:]  # pre-rounded F32R identity

    # ================= projection phase =================
    with ExitStack() as pctx:
        trig_pool = pctx.enter_context(tc.tile_pool(name="trig", bufs=1))
        xt_pool = pctx.enter_context(tc.tile_pool(name="xt", bufs=18))
        rope_pool = pctx.enter_context(tc.tile_pool(name="rope", bufs=2))
        vt_pool = pctx.enter_context(tc.tile_pool(name="vt", bufs=2))
        pt_ps = pctx.enter_context(tc.tile_pool(name="pt_ps", bufs=2, space="PSUM"))
        pj_ps = pctx.enter_context(tc.tile_pool(name="pj_ps", bufs=4, space="PSUM"))

        cosT_sb = trig_pool.tile([P, S], F32)
        sinT_sb = trig_pool.tile([P, S], F32)

        x_r = x_in.rearrange("(et p) s -> p et s", p=P)

        for sc in range(NQC):
            s0 = sc * 512
            # x^T tiles of this s-chunk arrive pre-transposed from the host;
            # chunk 0 was prefetched during the previous rep's attention
            xts = []
            if sc == 0 and use_prefetch:
                for et in range(ET):
                    xts.append(xpre[:, et, :])
            else:
                for et in range(ET):
                    xt_t = xt_pool.tile([P, 512], BF16, name="xts", tag="xts")
                    nc.sync.dma_start(xt_t[:], x_r[:, et, s0:s0 + 512])
                    xts.append(xt_t)
            if sc == 0:
                nc.sync.dma_start(cosT_sb[:], cosT_in[:])
                nc.sync.dma_start(sinT_sb[:], sinT_in[:])
            if sc == 2 and deferred is not None:
                # previous rep's final o_proj half: its AllGather has had a
                # full proj chunk of time to land, so the PE never stalls
                deferred()
                deferred = None

            # d6-outer matmul loop over resident xts tiles
            cos_c = cosT_sb[:, s0:s0 + 512]
            sin_c = sinT_sb[:, s0:s0 + 512]
            for d6 in range(6):
                pp = pj_ps.tile([P, 512], F32, name="pp", tag="pp")
                for et in range(ET):
                    if d6 < 4:
                        lhsT = wqT_sb[:, et, d6 * HD:(d6 + 1) * HD]
                    elif d6 == 4:
                        lhsT = wkT_sb[:, et, :]
                    else:
                        lhsT = wvT_sb[:, et, :]
                    nc.tensor.matmul(pp[:], lhsT, xts[et][:],
                                     start=(et == 0), stop=(et == ET - 1))
                if d6 < 5:
                    dst = (QT_sb[:, d6, s0:s0 + 512] if d6 < 4
                           else KT_sb[:, s0:s0 + 512])
                    t1 = rope_pool.tile([P, 512], F32, name="t1", tag="t1")
                    t2 = rope_pool.tile([P, 512], F32, name="t2", tag="t2")
                    nc.vector.tensor_tensor(t1[:], pp[:], cos_c,
                                            mybir.AluOpType.mult)
                    # sinT arrives with rows 0:64 pre-negated (host side)
                    nc.vector.tensor_tensor(t2[0:64, :], pp[64:128, :],
                                            sin_c[0:64, :],
                                            mybir.AluOpType.mult)
                    nc.vector.tensor_tensor(t2[64:128, :], pp[0:64, :],
                                            sin_c[64:128, :],
                                            mybir.AluOpType.mult)
                    nc.vector.tensor_tensor(dst[:], t1[:], t2[:],
                                            mybir.AluOpType.add)
                else:
                    vts = vt_pool.tile([P, 512], F32R, name="vts", tag="vts")
                    nc.scalar.copy(vts[:], pp[:])
                    for st4 in range(4):
                        pv_t = pt_ps.tile([P, 512], F32, name="pvt",
                                          tag="ptile")[:, 0:P]
                        nc.tensor.transpose(r32(pv_t[:]),
                                            vts[:, st4 * P:(st4 + 1) * P],
                                            identr)
                        nc.scalar.copy(V_sb[:, sc * 4 + st4, :], pv_t[:])

    # ================= attention + o_proj phase =================
    with ExitStack() as actx:
        ex_pool = actx.enter_context(tc.tile_pool(name="ex", bufs=8))
        dn_pool = actx.enter_context(tc.tile_pool(name="dn", bufs=2))
        sm_pool = actx.enter_context(tc.tile_pool(name="sm", bufs=2))
        bc_pool = actx.enter_context(tc.tile_pool(name="bc", bufs=2))
        oh_pool = actx.enter_context(tc.tile_pool(name="oh", bufs=3))
        sc_ps = actx.enter_context(tc.tile_pool(name="sc_ps", bufs=3, space="PSUM"))
        pv_ps = actx.enter_context(tc.tile_pool(name="pv_ps", bufs=2, space="PSUM"))
        dn_ps = actx.enter_context(tc.tile_pool(name="dn_ps", bufs=1, space="PSUM"))

        NHALF = min(AG_HALVES, NQC)
        cph = NQC // NHALF

        def emit_ag(h, hf):
            if NO_COLLECTIVE:
                for mt in range(4):
                    nc.sync.dma_start(
                        agout[h][hf][mt * P:(mt + 1) * P, :], agin[h][hf][:])
            else:
                nc.gpsimd.collective_compute(
                    "AllGather", mybir.AluOpType.bypass,
                    replica_groups=REPLICA_GROUPS,
                    ins=[agin[h][hf].opt()],
                    outs=[agout[h][hf].opt()])

        x_rA = x_in.rearrange("(et p) s -> p et s", p=P)
        bg = []  # background o_proj groups interleaved into kt slots
        for h in range(NHL):
            if h == NHL - 1 and do_prefetch:
                # pull the next rep's chunk-0 xT tiles in now, while the
                # DMA queues are quiet, so the rep boundary never starves
                for et in range(ET):
                    nc.sync.dma_start(xpre[:, et, :], x_rA[:, et, 0:512])
            if h > 0:
                bg += _oproj_groups(nc, h - 1, cph, NHALF, agout, af_pool,
                                    oo_ps, woT_sb, outAcc, out_d, hf=0)
            for qc in range(NQC):
                q0 = qc * 512
                nkt = 4 * qc + 4
                qT = QT_sb[:, h, q0:q0 + 512]
                pv = pv_ps.tile([P, 512], F32, name="pv", tag="pv")
                denacc = dn_pool.tile([P, 512], F16, name="denacc",
                                      tag="denacc")
                exs = [None] * nkt
                LAG = 4  # pv(kt-LAG) emitted after scores(kt): hides exp+mask
                def emit_pv(j, last):
                    nc.tensor.matmul(pv[:], V_sb[:, j, :], exs[j][:],
                                     start=(j == 0), stop=last)
                for kt in range(nkt):
                    ps = sc_ps.tile([P, 512], F32, name="ps", tag="ps")
                    nc.tensor.matmul(
                        ps[:], KT_sb[:, kt * P:(kt + 1) * P], qT,
                        start=True, stop=True)
                    ex = ex_pool.tile([P, 512], F16, name="ex", tag="ex")
                    exs[kt] = ex
                    nc.scalar.activation(ex[:], ps[:],
                                         mybir.ActivationFunctionType.Exp,
                                         scale=float(SCALE))
                    t = kt - 4 * qc
                    if t >= 0:
                        nc.vector.tensor_tensor(
                            ex[:], ex[:],
                            masks[:, t * 512:(t + 1) * 512],
                            mybir.AluOpType.mult)
                    # denominator: fp16 adds on DVE (frees the PE)
                    if kt == 0:
                        nc.vector.tensor_copy(denacc[:], ex[:])
                    else:
                        nc.vector.tensor_add(denacc[:], denacc[:], ex[:])
                    if kt >= LAG:
                        emit_pv(kt - LAG, last=False)
                    if h > 0 and qc == NQC - 1 and kt == 4:
                        # previous head's second-half AllGather has now had
                        # ~3 q-chunks of attention to land
                        bg += _oproj_groups(nc, h - 1, cph, NHALF, agout,
                                            af_pool, oo_ps, woT_sb, outAcc,
                                            out_d, hf=1)
                    if bg:
                        bg.pop(0)()
                for j in range(max(0, nkt - LAG), nkt):
                    emit_pv(j, last=(j == nkt - 1))
                # partition-reduce the fp16 denominator on the PE (1 matmul)
                pden = dn_ps.tile([P, 512], F32, name="pden", tag="pden")
                nc.tensor.matmul(pden[0:1, :], ones_red[:], denacc[:],
                                 start=True, stop=True)
                rec = sm_pool.tile([1, 512], F32, name="rec", tag="rec")
                nc.vector.reciprocal_approx_fast(out=rec[:], in_=pden[0:1, :])
                recr = sm_pool.tile([1, 512], F32R, name="recr", tag="recr")
                nc.vector.tensor_copy(recr[:], rec[:])
                pbc = sc_ps.tile([P, 512], F32, name="pbc", tag="ps")
                nc.tensor.matmul(pbc[:], ones_col[:], recr[:],
                                 start=True, stop=True)
                bcr = bc_pool.tile([P, 512], F32, name="bcr", tag="bcr")
                nc.vector.tensor_copy(bcr[:], pbc[:])
                outH = oh_pool.tile([P, 512], BF16, name="outH", tag="outH")
                nc.vector.tensor_tensor(outH[:], pv[:], bcr[:],
                                        mybir.AluOpType.mult)
                hf = qc // cph
                qh0 = (qc - hf * cph) * 512
                nc.sync.dma_start(agin[h][hf][:, qh0:qh0 + 512], outH[:])
                # ship this half as soon as its last q-chunk is done;
                # then emit the o_proj half of the previous head whose
                # AllGather has had a full half-head of attention to land
                if qc % cph == cph - 1:
                    emit_ag(h, hf)
        while bg:
            bg.pop(0)()
        _emit_oproj(nc, NHL - 1, NQC, cph, NHALF, agout, af_pool,
                    oo_ps, woT_sb, outAcc, out_d, only_hf=0)

    def deferred_tail():
        _emit_oproj(nc, NHL - 1, NQC, cph, NHALF, agout, af_pool,
                    oo_ps, woT_sb, outAcc, out_d, only_hf=NHALF - 1)
    return deferred_tail


# ======================= host side =======================

_CACHE = {}


def _get_program(S=2048, reps=1):
    key = (S, reps, AG_HALVES, NO_COLLECTIVE, DEN_LAG)
    if key not in _CACHE:
        _CACHE[key] = build_program(S=S, reps=reps)
    return _CACHE[key]


def make_in_maps(x, cos, sin, wq, wk, wv, wo):
    in_maps = []
    cosT = np.ascontiguousarray(cos.T.astype(np.float32))
    sinT = sin.T.astype(np.float32).copy()
    sinT[:HD // 2, :] *= -1.0   # fold rotate_half sign into the table
    sinT = np.ascontiguousarray(sinT)
    for c in range(8):
        b, g = c // 4, c % 4
        in_maps.append({
            "xT": np.ascontiguousarray(x[b].T.astype(ml_dtypes.bfloat16)),
            "wqT": np.ascontiguousarray(wq[g * DQ:(g + 1) * DQ, :].T.astype(ml_dtypes.bfloat16)),
            "wkT": np.ascontiguousarray(wk[g * HD:(g + 1) * HD, :].T.astype(ml_dtypes.bfloat16)),
            "wvT": np.ascontiguousarray(wv[g * HD:(g + 1) * HD, :].T.astype(ml_dtypes.bfloat16)),
            "woT": np.ascontiguousarray(
                wo[g * DQ:(g + 1) * DQ, :].T.astype(ml_dtypes.bfloat16)),
            "cosT": cosT,
            "sinT": sinT,
        })
    return in_maps


def assemble_output(results, B, S):
    out = np.empty((B, S, E), np.float32)
    for c in range(8):
        b, g = c // 4, c % 4
        out[b][:, g * DQ:(g + 1) * DQ] = results[c]["out"].T
    return out


# ---- inline SPMD runner (PJRT/axon), device-resident inputs ----

class SpmdRunner:
    def __init__(self, nc, n_cores):
        import jax
        from jax.sharding import Mesh, PartitionSpec
        from jax.experimental.shard_map import shard_map
        from concourse import bass2jax
        from concourse.bass2jax import _bass_exec_p, install_neuronx_cc_hook

        install_neuronx_cc_hook()
        self.jax = jax
        self.nc = nc
        self.n_cores = n_cores
        partition_name = (nc.partition_id_tensor.name
                          if nc.partition_id_tensor else None)
        in_names, out_names, out_avals = [], [], []
        zero_outs = []
        for alloc in nc.m.functions[0].allocations:
            if not isinstance(alloc, mybir.MemoryLocationSet):
                continue
            name = alloc.memorylocations[0].name
            if alloc.kind == "ExternalInput":
                if name != partition_name:
                    in_names.append(name)
            elif alloc.kind == "ExternalOutput":
                out_names.append(name)
                shape = tuple(alloc.tensor_shape)
                dtype = mybir.dt.np(alloc.dtype)
                out_avals.append(jax.core.ShapedArray(shape, dtype))
                zero_outs.append(np.zeros(shape, dtype))
        self.in_names, self.out_names = in_names, out_names
        self.out_avals, self.zero_outs = out_avals, zero_outs
        self.n_params = len(in_names)

        all_in = list(in_names) + list(out_names)
        if partition_name is not None:
            all_in.append(partition_name)

        def _body(*args):
            operands = list(args)
            if partition_name is not None:
                operands.append(bass2jax.partition_id_tensor())
            outs = _bass_exec_p.bind(
                *operands, out_avals=tuple(out_avals),
                in_names=tuple(all_in), out_names=tuple(out_names),
                lowering_input_output_aliases=(),
                sim_require_finite=True, sim_require_nnan=True, nc=nc)
            return tuple(outs)

        devices = jax.devices()[:n_cores]
        self.mesh = Mesh(np.asarray(devices), ("core",))
        n_outs = len(out_names)
        in_specs = (PartitionSpec("core"),) * (self.n_params + n_outs)
        out_specs = (PartitionSpec("core"),) * n_outs
        self.fn = jax.jit(
            shard_map(_body, mesh=self.mesh, in_specs=in_specs,
                      out_specs=out_specs, check_rep=False),
            keep_unused=True)
        self.dev_args = None

    def stage_inputs(self, in_maps):
        import jax
        from jax.sharding import PartitionSpec
        per_core = [[np.asarray(m[n]) for n in self.in_names] for m in in_maps]
        concat_in = [
            np.concatenate([per_core[c][i] for c in range(self.n_cores)], axis=0)
            for i in range(self.n_params)]
        concat_zeros = [
            np.zeros((self.n_cores * z.shape[0], *z.shape[1:]), z.dtype)
            for z in self.zero_outs]
        sharding = jax.sharding.NamedSharding(self.mesh, PartitionSpec("core"))
        self.dev_args = [jax.device_put(a, sharding)
                         for a in (*concat_in, *concat_zeros)]
        for a in self.dev_args:
            a.block_until_ready()

    def run(self):
        out_arrs = [np.asarray(o) for o in self.fn(*self.dev_args)]
        return [
            {n: out_arrs[i].reshape(self.n_cores, *self.out_avals[i].shape)[c]
             for i, n in enumerate(self.out_names)}
            for c in range(self.n_cores)]

    def time_exec(self, iters=30, warmup=3):
        import jax
        for _ in range(warmup):
            res = self.fn(*self.dev_args)
        jax.block_until_ready(res)
        t0 = time.perf_counter()
        for _ in range(iters):
            res = self.fn(*self.dev_args)
        jax.block_until_ready(res)
        t1 = time.perf_counter()
        return (t1 - t0) / iters * 1e9


_RUNNER_CACHE = {}


def get_runner(S=2048, reps=1):
    key = (S, reps, AG_HALVES, NO_COLLECTIVE, DEN_LAG)
    if key not in _RUNNER_CACHE:
        nc = _get_program(S=S, reps=reps)
        _RUNNER_CACHE[key] = SpmdRunner(nc, 8)
    return _RUNNER_CACHE[key]


def kernel(x, cos, sin, wq, wk, wv, wo):
    B, S, _ = x.shape
    runner = get_runner(S=S, reps=1)
    runner.stage_inputs(make_in_maps(x, cos, sin, wq, wk, wv, wo))
    results = runner.run()
    return assemble_output(results, B, S)


if __name__ == "__main__":
    # tiny self-test against a local numpy reference
    S = int(sys.argv[1]) if len(sys.argv) > 1 else 512
    rng = np.random.default_rng(0)
    B, H, HKV = 2, 16, 4
    x = rng.standard_normal((B, S, E), dtype=np.float32)
    cos = rng.random((S, HD), dtype=np.float32)
    sin = rng.random((S, HD), dtype=np.float32)
    sc = 0.02
    wq = (rng.standard_normal((H * HD, E), dtype=np.float32) * sc)
    wk = (rng.standard_normal((HKV * HD, E), dtype=np.float32) * sc)
    wv = (rng.standard_normal((HKV * HD, E), dtype=np.float32) * sc)
    wo = (rng.standard_normal((E, H * HD), dtype=np.float32) * sc)

    def ref(x, cos, sin, wq, wk, wv, wo):
        x64 = x.astype(np.float64)
        q = (x64 @ wq.T.astype(np.float64)).reshape(B, S, H, HD)
        k = (x64 @ wk.T.astype(np.float64)).reshape(B, S, HKV, HD)
        v = (x64 @ wv.T.astype(np.float64)).reshape(B, S, HKV, HD)

        def rot(t):
            return np.concatenate([-t[..., HD // 2:], t[..., :HD // 2]], -1)

        c = cos[:, None, :].astype(np.float64)
        s = sin[:, None, :].astype(np.float64)
        q = q * c + rot(q) * s
        k = k * c + rot(k) * s
        k = np.repeat(k, H // HKV, axis=2).transpose(0, 2, 1, 3)
        v = np.repeat(v, H // HKV, axis=2).transpose(0, 2, 1, 3)
        q = q.transpose(0, 2, 1, 3)
        scores = np.einsum("bhqd,bhkd->bhqk", q, k) / np.sqrt(HD)
        mask = np.tril(np.ones((S, S), bool))
        scores = np.where(mask, scores, -np.inf)
        scores -= scores.max(-1, keepdims=True)
        p = np.exp(scores)
        p /= p.sum(-1, keepdims=True)
        o = np.einsum("bhqk,bhkd->bhqd", p, v)
        o = o.transpose(0, 2, 1, 3).reshape(B, S, H * HD)
        return o @ wo.T.astype(np.float64)

    want = ref(x, cos, sin, wq, wk, wv, wo)
    got = kernel(x, cos, sin, wq, wk, wv, wo)
    err = np.abs(got - want).max() / np.abs(want).max()
    print(f"S={S}: rel err (absmax-relative) = {err:.3e}")
